# revision 37
# baseline (speedup 1.0000x reference)
"""Trainium2 Bass kernel for nn_AlignModel.

Computes out[b, j, i] = sigmoid(simp[b,j]·w_s + orig[b,i]·w_o + bias) where
orig/simp are the two halves of prop_state[b] ([B, 2S, D] -> [B,S,D] each),
w_o = W[0,:D], w_s = W[0,D:].

Sharding: data-parallel over batch B=8 across the 8 NeuronCores.  Host-side
staging per core (layout only -- all compute is on device):
  xot  [512, 2048] f16  = orig(b).T          (d-major, so PE can contract d)
  xs   [2048, 512] f16  = simp(b)
  wrep [128, 4, 128] f16: wrep[k,e,m] = w_o[e*128+k]  (stationary replicated
        along the PE output dim -> matmul broadcasts s_o to all partitions)
  wsbc [128, 512]  f16  = w_s replicated across partitions
  out  [2048, 2048] f16, host-upcast to f32.

The 2e-2 rel-err gate admits fp16 end to end (sigmoid outputs in (0,1):
~5e-4 rel err; fp16-input dots with f32 accumulation: ~1e-4 score error).
Per-core HBM traffic: 4.45 MiB in + 8.39 MiB out.

Engine schedule (from trace iteration; engines run disjoint jobs):
  - PE: psum_so[p,i] = b + sum_d w_o[d]*orig[i,d] via 4 bias seeds + 16
    K=128/N=512 fp16 matmuls (w_rep stationary).  s_o never materializes;
    the matmul does the reduction AND the 128-row broadcast.
  - DVE: simp dots only (fp16 mul at 2x + batched 4-tile reduce at 1x)
    into s_sb_mat columns -> always ahead of ScalarE's 2us/tile cadence.
  - ScalarE: ONLY the 16 sigmoid ACTIVATEs, [128,2048] PSUM->SBUF f16,
    bias port adds s_s[t*128+p].  ACT table preloaded by a dep-free dummy.
  - Load order on the sync queue (FIFO): xs group 0 -> xot e=0..3 -> xs
    groups 1-3; stores follow.  All per-partition descriptor lines are
    >=4KB except xs (1KB, layout-forced); small chunks measured
    ~100-150 GB/s vs ~400 GB/s at 4KB.
"""

import numpy as np

import concourse.mybir as mybir
from concourse import bacc, bass_utils
from concourse.tile import TileContext

P = 128          # partitions
D = 512          # feature dim
S = 2048         # sents
NT = S // P      # 16 row-tiles
NE = D // P      # 4 contraction chunks
SCH = 4          # simp tiles per load group
NSC = NT // SCH
OGROUPS = [1, 1, 2, 4, 4, 2, 1, 1]   # output row-tiles per store
NCORES = 8
F32 = mybir.dt.float32
F16 = mybir.dt.float16


def _kernel_body(tc, out, xot, xs, wrep, wsbc, bvec):
    nc = tc.nc
    # simp half: logical layout is partition-inner (j = n*P + p, so bias
    # columns drop out of the reduce), but the HOST pre-permutes rows so
    # HBM row p*NT+n holds simp row n*P+p -> per-partition descriptor
    # lines are 16KB contiguous (1KB lines measured ~250 GB/s vs ~400)
    xs_re = xs.rearrange("(p n) d -> p n d", n=NT)

    with (
        tc.tile_pool(name="consts", bufs=1) as cpool,
        tc.tile_pool(name="xin", bufs=1) as xpool,
        tc.tile_pool(name="scratch", bufs=3) as spool,
        tc.tile_pool(name="outbuf", bufs=1) as opool,
        tc.tile_pool(name="psum", bufs=1, space="PSUM") as ppool,
    ):
        # preload the sigmoid ACT table while DMAs run: dummy activation
        # whose only deps are two DVE memsets, so it issues almost at t=0.
        dummy = cpool.tile([1, 1], F32, tag="dummy")
        dummy_b = cpool.tile([1, 1], F32, tag="dummyb")
        nc.vector.memset(dummy, 0.0)
        nc.vector.memset(dummy_b, 0.0)
        nc.scalar.activation(dummy, dummy,
                             mybir.ActivationFunctionType.Sigmoid,
                             bias=dummy_b[:, 0:1])

        # tiny loads on the scalar HWDGE queue (empty early, so these land
        # well before the big sync-queue stream needs them)
        wrep_sb = cpool.tile([P, NE, P], F16, tag="wrep")
        wsbc_sb = cpool.tile([P, D], F16, tag="wsbc")
        b_sb = cpool.tile([1, 1], F32, tag="bsb")
        nc.scalar.dma_start(out=wrep_sb, in_=wrep)
        nc.scalar.dma_start(out=wsbc_sb, in_=wsbc)
        nc.scalar.dma_start(out=b_sb, in_=bvec)

        ones_row = cpool.tile([1, P], F16, tag="ones")
        nc.vector.memset(ones_row, 1.0)
        b_row = cpool.tile([1, 512], F16, tag="brow")
        nc.vector.memset(b_row, 0.0)
        nc.vector.tensor_scalar_add(b_row, b_row, b_sb)

        # --- input stream (sync queue, FIFO): xs g0, xot e0..3, xs rest ---
        xs_all = xpool.tile([P, NT, D], F16, tag="xs")
        nc.sync.dma_start(out=xs_all[:, 0:SCH, :], in_=xs_re[:, 0:SCH, :])
        xot_t = []
        for e in range(NE):
            xt = xpool.tile([P, S], F16, tag=f"xot{e}", name=f"xot{e}")
            nc.sync.dma_start(out=xt, in_=xot[e * P:(e + 1) * P, :])
            xot_t.append(xt)
        nc.sync.dma_start(out=xs_all[:, SCH:10, :], in_=xs_re[:, SCH:10, :])
        nc.sync.dma_start(out=xs_all[:, 10:NT, :], in_=xs_re[:, 10:NT, :])

        s_sb_mat = cpool.tile([P, NT], F32, tag="ssmat")  # s_s, col t
        sob_psum = ppool.tile([P, S], F32, tag="sob")     # b + s_o, all rows

        # --- PE: b seed, then accumulate w_o-weighted transposed orig ---
        for j in range(S // 512):
            nc.tensor.matmul(sob_psum[:, j * 512:(j + 1) * 512], ones_row,
                             b_row, start=True, stop=False)
        for e in range(NE):
            for j in range(S // 512):
                nc.tensor.matmul(sob_psum[:, j * 512:(j + 1) * 512],
                                 wrep_sb[:, e, :],
                                 xot_t[e][:, j * 512:(j + 1) * 512],
                                 start=False, stop=(e == NE - 1))

        # --- simp dots (DVE) + sigmoid row-blocks (ScalarE) + stores ---
        group_of_tile = []
        for gi, gsz in enumerate(OGROUPS):
            group_of_tile += [gi] * gsz
        group_start = np.cumsum([0] + OGROUPS).tolist()

        out_all = opool.tile([P, NT, S], F16, tag="oall")
        for g in range(NSC):
            prod = spool.tile([P, SCH, D], F16, tag="prod", name=f"ps{g}")
            for blk in range(SCH):
                nc.vector.tensor_mul(out=prod[:, blk, :],
                                     in0=xs_all[:, g * SCH + blk, :],
                                     in1=wsbc_sb)
            nc.vector.tensor_reduce(
                s_sb_mat[:, g * SCH:(g + 1) * SCH], prod,
                axis=mybir.AxisListType.X, op=mybir.AluOpType.add)
            for blk in range(SCH):
                t = g * SCH + blk
                if t == 0:
                    # split: the first half starts as soon as the first two
                    # PSUM banks have their final accumulation
                    for c0 in (0, 1024):
                        nc.scalar.activation(
                            out_all[:, 0, c0:c0 + 1024],
                            sob_psum[:, c0:c0 + 1024],
                            mybir.ActivationFunctionType.Sigmoid,
                            bias=s_sb_mat[:, 0:1], scale=1.0)
                else:
                    nc.scalar.activation(
                        out_all[:, t, :], sob_psum,
                        mybir.ActivationFunctionType.Sigmoid,
                        bias=s_sb_mat[:, t:t + 1],
                        scale=1.0,
                    )
                gi = group_of_tile[t]
                if t == group_start[gi] + OGROUPS[gi] - 1:
                    t0_g = group_start[gi]
                    gsz = OGROUPS[gi]
                    r0 = t0_g * P
                    if gsz == 1:
                        nc.sync.dma_start(out=out[r0:r0 + P, :],
                                          in_=out_all[:, t0_g, :])
                    else:
                        dst = out[r0:r0 + gsz * P, :].rearrange(
                            "(q p) i -> p q i", p=P)
                        nc.sync.dma_start(out=dst,
                                          in_=out_all[:, t0_g:t0_g + gsz, :])


def build_program():
    nc = bacc.Bacc(
        "TRN2",
        debug=False,
        target_bir_lowering=False,
        num_devices=NCORES,
    )
    xot = nc.dram_tensor("xot", [D, S], F16, kind="ExternalInput").ap()
    xs = nc.dram_tensor("xs", [S, D], F16, kind="ExternalInput").ap()
    wrep = nc.dram_tensor("wrep", [P, NE, P], F16, kind="ExternalInput").ap()
    wsbc = nc.dram_tensor("wsbc", [P, D], F16, kind="ExternalInput").ap()
    bvec = nc.dram_tensor("bvec", [1, 1], F32, kind="ExternalInput").ap()
    out = nc.dram_tensor("out", [S, S], F16, kind="ExternalOutput").ap()
    with TileContext(nc) as tc:
        _kernel_body(tc, out, xot, xs, wrep, wsbc, bvec)
    nc.compile()
    return nc


_PROGRAM = None


def _get_program():
    global _PROGRAM
    if _PROGRAM is None:
        _PROGRAM = build_program()
    return _PROGRAM


def make_in_maps(prop_state, W, b):
    prop = np.asarray(prop_state, dtype=np.float32).astype(np.float16)
    w = np.asarray(W, dtype=np.float32).reshape(2 * D).astype(np.float16)
    w_o, w_s = w[:D], w[D:]
    # wrep[k, e, m] = w_o[e*128 + k], replicated along m (PE output dim)
    wrep = np.ascontiguousarray(
        np.broadcast_to(w_o.reshape(NE, P).T[:, :, None], (P, NE, P)))
    wsbc = np.ascontiguousarray(np.broadcast_to(w_s[None, :], (P, D)))
    bv = np.ascontiguousarray(np.asarray(b, dtype=np.float32).reshape(1, 1))
    maps = []
    for i in range(NCORES):
        xot = np.ascontiguousarray(prop[i, :S].T)         # [512, 2048]
        # permute simp rows so HBM row p*NT+n = simp row n*P+p (contiguous
        # per-partition descriptor lines for the partition-inner layout)
        xs = np.ascontiguousarray(
            prop[i, S:].reshape(NT, P, D).transpose(1, 0, 2).reshape(S, D))
        maps.append({"xot": xot, "xs": xs, "wrep": wrep,
                     "wsbc": wsbc, "bvec": bv})
    return maps


def kernel(A, prop_state, W, b, _trace=False):
    nc = _get_program()
    in_maps = make_in_maps(prop_state, W, b)
    res = bass_utils.run_bass_kernel_spmd(
        nc, in_maps, core_ids=list(range(NCORES)), trace=_trace)
    out = np.stack([res.results[i]["out"] for i in range(NCORES)], axis=0)
    if _trace:
        kernel.last_results = res
    return out.astype(np.float32)
